# revision 43
# baseline (speedup 1.0000x reference)
"""BagRE segment-mean + classifier kernel for 8 Trainium2 NeuronCores.

Problem:  hidden [262144, 256] f32, sorted bag_id [262144] i64 with 8192 bags,
          W [128, 256], b [128]  ->  logits [8192, 128] f32
          logits = (segment_mean(hidden, bag_id) @ W.T) + b

Strategy (no collectives needed):
  bag_id is sorted, so rows for any bag range are contiguous.  Core k owns
  bags [1024k, 1024(k+1)).  Each core's bags are split into 8 blocks of 128
  bags; the host pads every block's rows to a common tile count (multiple of
  128 rows) so all 8 cores run the same static program (SPMD).

  Per 128-row tile the device builds a one-hot matrix A[row, bag] in bf16
  (DVE is_equal of an iota row vs the per-row relative bag id) and
  accumulates A.T @ X into PSUM [128 bags, 256] on the tensor engine.
  X is pre-split on the host into bf16 hi + lo halves (hi = bf16(x),
  lo = bf16(x - hi)) so the PE streams at full rate; hi+lo matmuls
  accumulate into the same f32 PSUM, recovering ~16+ mantissa bits.

  Per block: copy sums to SBUF, PE-transpose to [H, bags] layout, then the
  classifier GEMM out[g, c] = sum_h sums[g, h] * W[c, h] in f32, and a fused
  DVE op applies the per-bag 1/count scale (host-computed from bag_id) plus
  the bias broadcast.  Output shards are concatenated on the host.
"""

import os
import sys
import types
import bisect
import contextlib
import numpy as np

try:
    import concourse.bass as bass  # noqa: F401
except Exception:  # pragma: no cover
    sys.path.insert(0, "/opt/trn_rl_repo")

import ml_dtypes
import concourse.bass as bass
import concourse.tile as tile
from concourse import mybir, bacc, masks
from concourse.bass_utils import run_bass_kernel_spmd

BF16 = ml_dtypes.bfloat16

N = 262144
H = 256
C = 128
NUM_BAGS = 8192
NCORES = 8
BLOCK_BAGS = 128                 # bags per PSUM block (= PE output partitions)
BLOCKS_PER_CORE = NUM_BAGS // BLOCK_BAGS // NCORES   # 8


def _pick_ch(T):
    # small chunks keep PE stalls well under the ~3.4us HAM re-throttle
    # window, so the tensor engine stays at 2.4 GHz through DMA waits
    for ch in (16, 8, 4, 2, 1):
        if T % ch == 0:
            return ch
    return 1

LAST_RESULTS = None              # BassKernelResults of the most recent run

_prog_cache = {}


def _install_ntff_shim():
    """Register the axon NTFF profiling hook so trace=True works.

    The agent image's ``antenv`` package lacks ``axon_hooks``; provide an
    in-process stand-in and wire it to the ctypes hook in trn_boot.
    Returns True if profiling is available.
    """
    try:
        from antenv.axon_hooks import get_axon_ntff_profile_hook  # noqa: F401
        return True
    except Exception:
        pass
    try:
        import antenv
        from trn_agent_boot.trn_boot import _ntff_profile_via_ctypes

        hook = _ntff_profile_via_ctypes("/opt/axon/libaxon_pjrt.so")
        if hook is None:
            return False
        mod = types.ModuleType("antenv.axon_hooks")
        mod._hook = hook
        mod.get_axon_ntff_profile_hook = lambda: mod._hook
        mod.set_axon_ntff_profile_hook = lambda h: setattr(mod, "_hook", h)
        sys.modules["antenv.axon_hooks"] = mod
        antenv.axon_hooks = mod
        # upload_artifacts needs a writable artifact bucket that this
        # container may not have; make it best-effort.
        import concourse.bass_utils as bu

        orig_upload = bu.upload_artifacts

        def _safe_upload(tmpdir):
            try:
                return orig_upload(tmpdir)
            except Exception:
                return tmpdir

        bu.upload_artifacts = _safe_upload
        return True
    except Exception:
        return False


def _build_program(pos_tblks: tuple):
    """One SPMD program per core: 8 blocks, pos_tblks[j] 128-row tiles each."""
    T = sum(pos_tblks)                   # 128-row tiles per core
    offs = [0]
    for tb in pos_tblks:
        offs.append(offs[-1] + tb)
    CH = _pick_ch(T)
    n_chunks = T // CH
    f32 = mybir.dt.float32
    bf16 = mybir.dt.bfloat16

    f16 = mybir.dt.float16
    nc = bacc.Bacc(trn_type="TRN2", target_bir_lowering=False, debug=False)
    hid = nc.dram_tensor("hid", [n_chunks, 128, CH * H], f16,
                         kind="ExternalInput").ap()
    # packed per-partition consts: [relT (T) | wt0 (C) | wt1 (C) | b (C) | recip]
    CW = T + 3 * C + BLOCKS_PER_CORE
    cst = nc.dram_tensor("cst", [128, CW], f32, kind="ExternalInput").ap()
    iota = nc.dram_tensor("iota", [128, BLOCK_BAGS], f16,
                          kind="ExternalInput").ap()
    out = nc.dram_tensor("out", [BLOCKS_PER_CORE, 128, C], f32,
                         kind="ExternalOutput").ap()

    with tile.TileContext(nc) as tc:
        with contextlib.ExitStack() as ctx:
            consts = ctx.enter_context(tc.tile_pool(name="consts", bufs=1))
            hid_pool = ctx.enter_context(tc.tile_pool(name="hid", bufs=8))
            oh_pool = ctx.enter_context(tc.tile_pool(name="onehot", bufs=6))
            psum_s = ctx.enter_context(
                tc.tile_pool(name="psum_s", bufs=3, space="PSUM"))
            sums_pool = ctx.enter_context(tc.tile_pool(name="sums", bufs=2))
            psum_t = ctx.enter_context(
                tc.tile_pool(name="psum_t", bufs=2, space="PSUM"))
            sumsT_pool = ctx.enter_context(tc.tile_pool(name="sumsT", bufs=4))
            psum_o = ctx.enter_context(
                tc.tile_pool(name="psum_o", bufs=2, space="PSUM"))
            out_pool = ctx.enter_context(tc.tile_pool(name="outsb", bufs=2))

            cst_t = consts.tile([128, CW], f32)
            cst_dma = nc.sync.dma_start(cst_t[:], cst[:])
            iota_t = consts.tile([128, BLOCK_BAGS], f16)
            iota_dma = nc.sync.dma_start(iota_t[:], iota[:])
            relT_t = cst_t[:, 0:T]
            wt_t = [cst_t[:, T + q * C:T + (q + 1) * C] for q in range(2)]
            b_t = cst_t[:, T + 2 * C:T + 3 * C]
            recip_t = cst_t[:, T + 3 * C:T + 3 * C + BLOCKS_PER_CORE]
            ident_t = consts.tile([128, 128], f32)
            masks.make_identity(nc, ident_t[:])

            # pre-warm the PE during the initial DMA fill so the first
            # streaming matmuls run at 2.4 GHz instead of the cold 1.2
            warm_in = consts.tile([128, 128], f16)
            nc.vector.memset(warm_in[:], 0.0)
            warm_ps = psum_s.tile([128, H], f32, name="warm", tag="psum_s")
            for w in range(24):
                nc.tensor.matmul(warm_ps[:, 0:128], warm_in[:], warm_in[:],
                                 start=True, stop=True)

            def finalize_steps(j, psum_fin):
                """Yield one finalize step of block j at a time so the PE ops
                interleave with the next block's streaming matmuls."""
                sums_t = sums_pool.tile([128, H], f32, name="sums",
                                        tag="sums")
                nc.scalar.copy(sums_t[:], psum_fin[:])
                yield
                sT = []
                for q in range(2):
                    p_t = psum_t.tile([128, 128], f32, name="psum_t",
                                      tag="psum_t")
                    nc.tensor.transpose(
                        p_t[:], sums_t[:, q * 128:(q + 1) * 128], ident_t[:])
                    s_t = sumsT_pool.tile([128, 128], f32, name="sumsT",
                                          tag="sumsT")
                    nc.scalar.copy(s_t[:], p_t[:])
                    sT.append(s_t)
                    yield
                po_t = psum_o.tile([128, C], f32, name="psum_o", tag="psum_o")
                nc.tensor.matmul(po_t[:], sT[0][:], wt_t[0],
                                 start=True, stop=False)
                yield
                nc.tensor.matmul(po_t[:], sT[1][:], wt_t[1],
                                 start=False, stop=True)
                yield
                ob_t = out_pool.tile([128, C], f32, name="outsb", tag="outsb")
                # ob = po * recip[:, j] + b
                nc.vector.scalar_tensor_tensor(
                    ob_t[:], po_t[:], recip_t[:, j:j + 1], b_t,
                    mybir.AluOpType.mult, mybir.AluOpType.add)
                nc.sync.dma_start(out[j], ob_t[:])
                yield

            psum_cur = None
            pending_fin = None
            for c in range(n_chunks):
                hid_t = hid_pool.tile([128, CH * H], f16, tag="hid")
                dma_eng = nc.sync if (c % 2 == 0) else nc.gpsimd
                chunk_dma = dma_eng.dma_start(hid_t[:], hid[c])
                if c == 1:
                    # the gpsimd-issued chunk must not race the consts DMA
                    # into the HW queues (the tiny consts transfer would sit
                    # behind it and stall the first one-hot)
                    bass._add_dep_helper(
                        chunk_dma.ins, iota_dma.ins, sync=True,
                        reason="consts-before-stream")

                for s in range(CH):
                    t = c * CH + s
                    j = bisect.bisect_right(offs, t) - 1
                    i = t - offs[j]
                    t_blk = pos_tblks[j]

                    a_t = oh_pool.tile([128, BLOCK_BAGS], f16, tag="onehot")
                    nc.vector.tensor_scalar(
                        a_t[:], iota_t[:], relT_t[:, t:t + 1], None,
                        mybir.AluOpType.is_equal)

                    if i == 0:
                        psum_cur = psum_s.tile([128, H], f32, tag="psum_s")
                    nc.tensor.matmul(
                        psum_cur[:], a_t[:], hid_t[:, s * H:(s + 1) * H],
                        start=(i == 0), stop=(i == t_blk - 1))

                    if i == t_blk - 1:
                        for _ in finalize_steps(j, psum_cur):
                            pass
    nc.compile()
    return nc


def kernel(hidden, W, b, bag_id):
    global LAST_RESULTS
    hidden = np.asarray(hidden, dtype=np.float32)
    W = np.asarray(W, dtype=np.float32)
    b = np.asarray(b, dtype=np.float32)
    bag_id = np.asarray(bag_id)

    n, h = hidden.shape
    assert (n, h) == (N, H) and W.shape == (C, H)

    # ---- host-side index preprocessing -------------------------------
    counts = np.bincount(bag_id.astype(np.int64), minlength=NUM_BAGS)
    recip_all = (1.0 / np.maximum(counts, 1)).astype(np.float32)

    nblocks = NUM_BAGS // BLOCK_BAGS                     # 64
    edges = np.searchsorted(bag_id, np.arange(0, NUM_BAGS + 1, BLOCK_BAGS))
    blk_len = np.diff(edges)                             # rows per block
    tiles_per_blk = np.maximum(1, -(-blk_len // 128))    # [64]
    # per block POSITION (same program on all 8 cores): max over cores
    pos_tblks = tiles_per_blk.reshape(NCORES, BLOCKS_PER_CORE).max(axis=0)
    # total tiles per core must divide the 8-tile DMA chunk
    pos_tblks[-1] += (-int(pos_tblks.sum())) % 16
    pos_tblks = tuple(int(x) for x in pos_tblks)
    T = sum(pos_tblks)
    offs = np.concatenate([[0], np.cumsum(pos_tblks)])

    # padded per-(core, position) rows + relative bag ids, flattened to the
    # per-core tile stream layout [NCORES, T*128, ...]
    xp16 = np.zeros((NCORES, T * 128, H), dtype=np.float16)
    rel = np.full((NCORES, T * 128), -1.0, dtype=np.float32)
    for bidx in range(nblocks):
        k, j = divmod(bidx, BLOCKS_PER_CORE)
        s, e = int(edges[bidx]), int(edges[bidx + 1])
        ln = e - s
        r0 = int(offs[j]) * 128
        if ln:
            xp16[k, r0:r0 + ln] = hidden[s:e]
            rel[k, r0:r0 + ln] = (bag_id[s:e] - bidx * BLOCK_BAGS).astype(
                np.float32)

    CH = _pick_ch(T)
    n_chunks = T // CH
    wt_np = np.ascontiguousarray(W.T).reshape(2, 128, C)
    b_np = np.tile(b, (128, 1)).astype(np.float32)
    iota_np = np.tile(np.arange(BLOCK_BAGS, dtype=np.float16), (128, 1))

    def chunkify(arr):   # [T*128, H] f16 -> [n_chunks, 128, CH*H]
        a = arr.reshape(T, 128, H).reshape(n_chunks, CH, 128, H)
        return np.ascontiguousarray(a.transpose(0, 2, 1, 3)).reshape(
            n_chunks, 128, CH * H)

    in_maps = []
    for k in range(NCORES):
        relc = rel[k].reshape(T, 128)
        recc = recip_all[k * 1024:(k + 1) * 1024].reshape(
            BLOCKS_PER_CORE, 128).T
        cst_np = np.concatenate(
            [relc.T, wt_np[0], wt_np[1], b_np, recc],
            axis=1).astype(np.float32)
        in_maps.append({
            "hid": chunkify(xp16[k]),
            "cst": np.ascontiguousarray(cst_np),
            "iota": iota_np,
        })

    # ---- build / fetch program ---------------------------------------
    if pos_tblks not in _prog_cache:
        _prog_cache[pos_tblks] = _build_program(pos_tblks)
    nc = _prog_cache[pos_tblks]

    trace = False
    if os.environ.get("BASS_TRACE"):
        trace = _install_ntff_shim()

    res = run_bass_kernel_spmd(nc, in_maps, core_ids=list(range(NCORES)),
                               trace=trace)
    LAST_RESULTS = res

    out = np.concatenate(
        [res.results[k]["out"].reshape(1024, C) for k in range(NCORES)],
        axis=0)
    return out


# revision 44
# speedup vs baseline: 1.1263x; 1.1263x over previous
"""BagRE segment-mean + classifier kernel for 8 Trainium2 NeuronCores.

Problem:  hidden [262144, 256] f32, sorted bag_id [262144] i64 with 8192 bags,
          W [128, 256], b [128]  ->  logits [8192, 128] f32
          logits = (segment_mean(hidden, bag_id) @ W.T) + b

Strategy (no collectives needed):
  bag_id is sorted, so rows for any bag range are contiguous.  Core k owns
  bags [1024k, 1024(k+1)).  Each core's bags are split into 8 blocks of 128
  bags; the host pads every block's rows to a common tile count (multiple of
  128 rows) so all 8 cores run the same static program (SPMD).

  Per 128-row tile the device builds a one-hot matrix A[row, bag] in bf16
  (DVE is_equal of an iota row vs the per-row relative bag id) and
  accumulates A.T @ X into PSUM [128 bags, 256] on the tensor engine.
  X is pre-split on the host into bf16 hi + lo halves (hi = bf16(x),
  lo = bf16(x - hi)) so the PE streams at full rate; hi+lo matmuls
  accumulate into the same f32 PSUM, recovering ~16+ mantissa bits.

  Per block: copy sums to SBUF, PE-transpose to [H, bags] layout, then the
  classifier GEMM out[g, c] = sum_h sums[g, h] * W[c, h] in f32, and a fused
  DVE op applies the per-bag 1/count scale (host-computed from bag_id) plus
  the bias broadcast.  Output shards are concatenated on the host.
"""

import os
import sys
import types
import bisect
import contextlib
import numpy as np

try:
    import concourse.bass as bass  # noqa: F401
except Exception:  # pragma: no cover
    sys.path.insert(0, "/opt/trn_rl_repo")

import ml_dtypes
import concourse.bass as bass
import concourse.tile as tile
from concourse import mybir, bacc, masks
from concourse.bass_utils import run_bass_kernel_spmd

BF16 = ml_dtypes.bfloat16

N = 262144
H = 256
C = 128
NUM_BAGS = 8192
NCORES = 8
BLOCK_BAGS = 128                 # bags per PSUM block (= PE output partitions)
BLOCKS_PER_CORE = NUM_BAGS // BLOCK_BAGS // NCORES   # 8


def _pick_ch(T):
    # small chunks keep PE stalls well under the ~3.4us HAM re-throttle
    # window, so the tensor engine stays at 2.4 GHz through DMA waits
    for ch in (8, 4, 2, 1):
        if T % ch == 0:
            return ch
    return 1

LAST_RESULTS = None              # BassKernelResults of the most recent run

_prog_cache = {}


def _install_ntff_shim():
    """Register the axon NTFF profiling hook so trace=True works.

    The agent image's ``antenv`` package lacks ``axon_hooks``; provide an
    in-process stand-in and wire it to the ctypes hook in trn_boot.
    Returns True if profiling is available.
    """
    try:
        from antenv.axon_hooks import get_axon_ntff_profile_hook  # noqa: F401
        return True
    except Exception:
        pass
    try:
        import antenv
        from trn_agent_boot.trn_boot import _ntff_profile_via_ctypes

        hook = _ntff_profile_via_ctypes("/opt/axon/libaxon_pjrt.so")
        if hook is None:
            return False
        mod = types.ModuleType("antenv.axon_hooks")
        mod._hook = hook
        mod.get_axon_ntff_profile_hook = lambda: mod._hook
        mod.set_axon_ntff_profile_hook = lambda h: setattr(mod, "_hook", h)
        sys.modules["antenv.axon_hooks"] = mod
        antenv.axon_hooks = mod
        # upload_artifacts needs a writable artifact bucket that this
        # container may not have; make it best-effort.
        import concourse.bass_utils as bu

        orig_upload = bu.upload_artifacts

        def _safe_upload(tmpdir):
            try:
                return orig_upload(tmpdir)
            except Exception:
                return tmpdir

        bu.upload_artifacts = _safe_upload
        return True
    except Exception:
        return False


def _build_program(pos_tblks: tuple):
    """One SPMD program per core: 8 blocks, pos_tblks[j] 128-row tiles each."""
    T = sum(pos_tblks)                   # 128-row tiles per core
    offs = [0]
    for tb in pos_tblks:
        offs.append(offs[-1] + tb)
    CH = _pick_ch(T)
    n_chunks = T // CH
    f32 = mybir.dt.float32
    bf16 = mybir.dt.bfloat16

    f16 = mybir.dt.float16
    nc = bacc.Bacc(trn_type="TRN2", target_bir_lowering=False, debug=False)
    hid = nc.dram_tensor("hid", [n_chunks, 128, CH * H], f16,
                         kind="ExternalInput").ap()
    # packed per-partition consts: [relT (T) | wt0 (C) | wt1 (C) | b (C) | recip]
    CW = T + 3 * C + BLOCKS_PER_CORE
    cst = nc.dram_tensor("cst", [128, CW], f32, kind="ExternalInput").ap()
    iota = nc.dram_tensor("iota", [128, BLOCK_BAGS], f16,
                          kind="ExternalInput").ap()
    out = nc.dram_tensor("out", [BLOCKS_PER_CORE, 128, C], f32,
                         kind="ExternalOutput").ap()

    with tile.TileContext(nc) as tc:
        with contextlib.ExitStack() as ctx:
            consts = ctx.enter_context(tc.tile_pool(name="consts", bufs=1))
            hid_pool = ctx.enter_context(tc.tile_pool(name="hid", bufs=8))
            oh_pool = ctx.enter_context(tc.tile_pool(name="onehot", bufs=6))
            psum_s = ctx.enter_context(
                tc.tile_pool(name="psum_s", bufs=3, space="PSUM"))
            sums_pool = ctx.enter_context(tc.tile_pool(name="sums", bufs=2))
            psum_t = ctx.enter_context(
                tc.tile_pool(name="psum_t", bufs=2, space="PSUM"))
            sumsT_pool = ctx.enter_context(tc.tile_pool(name="sumsT", bufs=4))
            psum_o = ctx.enter_context(
                tc.tile_pool(name="psum_o", bufs=2, space="PSUM"))
            out_pool = ctx.enter_context(tc.tile_pool(name="outsb", bufs=2))

            cst_t = consts.tile([128, CW], f32)
            cst_dma = nc.sync.dma_start(cst_t[:], cst[:])
            iota_t = consts.tile([128, BLOCK_BAGS], f16)
            iota_dma = nc.sync.dma_start(iota_t[:], iota[:])
            relT_t = cst_t[:, 0:T]
            wt_t = [cst_t[:, T + q * C:T + (q + 1) * C] for q in range(2)]
            b_t = cst_t[:, T + 2 * C:T + 3 * C]
            recip_t = cst_t[:, T + 3 * C:T + 3 * C + BLOCKS_PER_CORE]
            ident_t = consts.tile([128, 128], f32)
            masks.make_identity(nc, ident_t[:])

            # pre-warm the PE during the initial DMA fill so the first
            # streaming matmuls run at 2.4 GHz instead of the cold 1.2
            warm_in = consts.tile([128, 128], f16)
            nc.vector.memset(warm_in[:], 0.0)
            warm_ps = psum_s.tile([128, H], f32, name="warm", tag="psum_s")
            for w in range(24):
                nc.tensor.matmul(warm_ps[:, 0:128], warm_in[:], warm_in[:],
                                 start=True, stop=True)

            def finalize_steps(j, psum_fin):
                """Yield one finalize step of block j at a time so the PE ops
                interleave with the next block's streaming matmuls."""
                sums_t = sums_pool.tile([128, H], f32, name="sums",
                                        tag="sums")
                nc.scalar.copy(sums_t[:], psum_fin[:])
                yield
                sT = []
                for q in range(2):
                    p_t = psum_t.tile([128, 128], f32, name="psum_t",
                                      tag="psum_t")
                    nc.tensor.transpose(
                        p_t[:], sums_t[:, q * 128:(q + 1) * 128], ident_t[:])
                    s_t = sumsT_pool.tile([128, 128], f32, name="sumsT",
                                          tag="sumsT")
                    nc.scalar.copy(s_t[:], p_t[:])
                    sT.append(s_t)
                    yield
                po_t = psum_o.tile([128, C], f32, name="psum_o", tag="psum_o")
                nc.tensor.matmul(po_t[:], sT[0][:], wt_t[0],
                                 start=True, stop=False)
                yield
                nc.tensor.matmul(po_t[:], sT[1][:], wt_t[1],
                                 start=False, stop=True)
                yield
                ob_t = out_pool.tile([128, C], f32, name="outsb", tag="outsb")
                # ob = po * recip[:, j] + b
                nc.vector.scalar_tensor_tensor(
                    ob_t[:], po_t[:], recip_t[:, j:j + 1], b_t,
                    mybir.AluOpType.mult, mybir.AluOpType.add)
                nc.sync.dma_start(out[j], ob_t[:])
                yield

            psum_cur = None
            pending_fin = None
            for c in range(n_chunks):
                hid_t = hid_pool.tile([128, CH * H], f16, tag="hid")
                dma_eng = nc.sync if (c % 2 == 0) else nc.gpsimd
                chunk_dma = dma_eng.dma_start(hid_t[:], hid[c])
                if c == 1:
                    # the gpsimd-issued chunk must not race the consts DMA
                    # into the HW queues (the tiny consts transfer would sit
                    # behind it and stall the first one-hot)
                    bass._add_dep_helper(
                        chunk_dma.ins, iota_dma.ins, sync=True,
                        reason="consts-before-stream")

                for s in range(CH):
                    t = c * CH + s
                    j = bisect.bisect_right(offs, t) - 1
                    i = t - offs[j]
                    t_blk = pos_tblks[j]

                    a_t = oh_pool.tile([128, BLOCK_BAGS], f16, tag="onehot")
                    nc.vector.tensor_scalar(
                        a_t[:], iota_t[:], relT_t[:, t:t + 1], None,
                        mybir.AluOpType.is_equal)

                    if i == 0:
                        psum_cur = psum_s.tile([128, H], f32, tag="psum_s")
                    nc.tensor.matmul(
                        psum_cur[:], a_t[:], hid_t[:, s * H:(s + 1) * H],
                        start=(i == 0), stop=(i == t_blk - 1))

                    if i == t_blk - 1:
                        for _ in finalize_steps(j, psum_cur):
                            pass
    nc.compile()
    return nc


def kernel(hidden, W, b, bag_id):
    global LAST_RESULTS
    hidden = np.asarray(hidden, dtype=np.float32)
    W = np.asarray(W, dtype=np.float32)
    b = np.asarray(b, dtype=np.float32)
    bag_id = np.asarray(bag_id)

    n, h = hidden.shape
    assert (n, h) == (N, H) and W.shape == (C, H)

    # ---- host-side index preprocessing -------------------------------
    counts = np.bincount(bag_id.astype(np.int64), minlength=NUM_BAGS)
    recip_all = (1.0 / np.maximum(counts, 1)).astype(np.float32)

    nblocks = NUM_BAGS // BLOCK_BAGS                     # 64
    edges = np.searchsorted(bag_id, np.arange(0, NUM_BAGS + 1, BLOCK_BAGS))
    blk_len = np.diff(edges)                             # rows per block
    tiles_per_blk = np.maximum(1, -(-blk_len // 128))    # [64]
    # per block POSITION (same program on all 8 cores): max over cores
    pos_tblks = tiles_per_blk.reshape(NCORES, BLOCKS_PER_CORE).max(axis=0)
    # total tiles per core must divide the 8-tile DMA chunk
    pos_tblks[-1] += (-int(pos_tblks.sum())) % 8
    pos_tblks = tuple(int(x) for x in pos_tblks)
    T = sum(pos_tblks)
    offs = np.concatenate([[0], np.cumsum(pos_tblks)])

    # padded per-(core, position) rows + relative bag ids, flattened to the
    # per-core tile stream layout [NCORES, T*128, ...]
    xp16 = np.zeros((NCORES, T * 128, H), dtype=np.float16)
    rel = np.full((NCORES, T * 128), -1.0, dtype=np.float32)
    for bidx in range(nblocks):
        k, j = divmod(bidx, BLOCKS_PER_CORE)
        s, e = int(edges[bidx]), int(edges[bidx + 1])
        ln = e - s
        r0 = int(offs[j]) * 128
        if ln:
            xp16[k, r0:r0 + ln] = hidden[s:e]
            rel[k, r0:r0 + ln] = (bag_id[s:e] - bidx * BLOCK_BAGS).astype(
                np.float32)

    CH = _pick_ch(T)
    n_chunks = T // CH
    wt_np = np.ascontiguousarray(W.T).reshape(2, 128, C)
    b_np = np.tile(b, (128, 1)).astype(np.float32)
    iota_np = np.tile(np.arange(BLOCK_BAGS, dtype=np.float16), (128, 1))

    def chunkify(arr):   # [T*128, H] f16 -> [n_chunks, 128, CH*H]
        a = arr.reshape(T, 128, H).reshape(n_chunks, CH, 128, H)
        return np.ascontiguousarray(a.transpose(0, 2, 1, 3)).reshape(
            n_chunks, 128, CH * H)

    in_maps = []
    for k in range(NCORES):
        relc = rel[k].reshape(T, 128)
        recc = recip_all[k * 1024:(k + 1) * 1024].reshape(
            BLOCKS_PER_CORE, 128).T
        cst_np = np.concatenate(
            [relc.T, wt_np[0], wt_np[1], b_np, recc],
            axis=1).astype(np.float32)
        in_maps.append({
            "hid": chunkify(xp16[k]),
            "cst": np.ascontiguousarray(cst_np),
            "iota": iota_np,
        })

    # ---- build / fetch program ---------------------------------------
    if pos_tblks not in _prog_cache:
        _prog_cache[pos_tblks] = _build_program(pos_tblks)
    nc = _prog_cache[pos_tblks]

    trace = False
    if os.environ.get("BASS_TRACE"):
        trace = _install_ntff_shim()

    res = run_bass_kernel_spmd(nc, in_maps, core_ids=list(range(NCORES)),
                               trace=trace)
    LAST_RESULTS = res

    out = np.concatenate(
        [res.results[k]["out"].reshape(1024, C) for k in range(NCORES)],
        axis=0)
    return out
